# revision 12
# baseline (speedup 1.0000x reference)
"""Causal self-attention (B=2, T=2048, D_in=1152, D=1024, H=16) on 8 trn2 cores.

Sharding: 2-way data parallel over batch x 4-way tensor parallel over heads.
Core c handles batch b = c//4 and heads [4g, 4g+4) with g = c%4.

Per-core dataflow (all matmuls in float32r, ~fp32 precision at bf16 speed):
  QT = (Wq_g)^T @ xp[b]^T   -> [256, 2048]   (head dims on partitions)
  KT likewise; V = xp[b] @ Wv_g in natural [T, 256] layout (T on partitions),
  stored with a ones-column per head: Vh~ = [V_h | 1] as [128, 16, 4, 65].
  Scores transposed: ST[k, q] = K Q^T built per 128-row k-block so softmax
  denominators come free: OT~ = Vh~^T @ exp(ST/8) accumulates [65, 512] in
  PSUM where row 64 is the softmax row-sum. exp is unshifted (scores are
  N(0,1) after scaling - safe in fp32). Causal mask = 0/1 multiply after exp
  on diagonal blocks only; fully-masked blocks are skipped.
  Normalize via reciprocal + rank-1 (K=32 zero-padded) broadcast matmul,
  write into OT_all [256, 2048], then Y_partial = OT_all^T @ Wp_g.
Host sums the 4 partial Y per batch and adds bp.
"""

import numpy as np

import concourse.bass as bass
import concourse.mybir as mybir
import concourse.tile as tile
from concourse import bacc
from concourse.bass_utils import run_bass_kernel_spmd

F32 = mybir.dt.float32
F32R = mybir.dt.float32r
AF = mybir.ActivationFunctionType
MUL = mybir.AluOpType.mult

B, T, DIN, D, H = 2, 2048, 1152, 1024, 16
HD = D // H           # 64 head dim
HLOC = 4              # heads per core
DLOC = HLOC * HD      # 256 local model dims
KC = DIN // 128       # 9 contraction chunks for projections
NT = T // 512         # 4 column tiles of 512
QC = T // 128         # 16 row chunks of 128
SCALE = 1.0 / np.sqrt(np.float32(HD))

_CACHE = {}


def _build():
    nc = bacc.Bacc(None)

    xpt = nc.dram_tensor("xpt", [DIN, T], F32, kind="ExternalInput")
    wq = nc.dram_tensor("wq", [DIN, DLOC], F32, kind="ExternalInput")
    wk = nc.dram_tensor("wk", [DIN, DLOC], F32, kind="ExternalInput")
    wv = nc.dram_tensor("wv", [DIN, DLOC], F32, kind="ExternalInput")
    bq = nc.dram_tensor("bq", [DLOC], F32, kind="ExternalInput")
    bk = nc.dram_tensor("bk", [DLOC], F32, kind="ExternalInput")
    bvp = nc.dram_tensor("bvp", [32, DLOC], F32, kind="ExternalInput")
    wp = nc.dram_tensor("wp", [DLOC, D], F32, kind="ExternalInput")
    mask = nc.dram_tensor("mask", [128, 4, 1024], F32, kind="ExternalInput")
    ones = nc.dram_tensor("ones", [32, 128], F32, kind="ExternalInput")
    onesm = nc.dram_tensor("onesm", [128, 4, 64], F32, kind="ExternalInput")
    vinit = nc.dram_tensor("vinit", [128, QC, HLOC, HD + 1], F32, kind="ExternalInput")
    onesr = nc.dram_tensor("onesr", [128, 512], F32, kind="ExternalInput")
    y = nc.dram_tensor("y", [T, D], F32, kind="ExternalOutput")

    xpt_r = xpt.rearrange("(ko p) t -> p ko t", p=128)
    wq_r = wq.rearrange("(ko p) d -> p ko d", p=128)
    wk_r = wk.rearrange("(ko p) d -> p ko d", p=128)
    wv_r = wv.rearrange("(ko p) d -> p ko d", p=128)
    wp_r = wp.rearrange("(c p) n -> p c n", p=128)
    bq_r = bq.rearrange("(m p) -> p m", p=128)
    bk_r = bk.rearrange("(m p) -> p m", p=128)

    with tile.TileContext(nc) as tc:
        with (
            tc.tile_pool(name="const", bufs=1) as cpool,
            tc.tile_pool(name="work", bufs=2) as wpool,
            tc.tile_pool(name="exp", bufs=4) as epool,
            tc.tile_pool(name="psB", bufs=3, space="PSUM") as psB,
            tc.tile_pool(name="psC", bufs=2, space="PSUM") as psC,
            nc.allow_low_precision(reason="float32r matmul pipeline"),
        ):
            t_wq = cpool.tile([128, KC, DLOC], F32R, tag="t_wq")
            t_wk = cpool.tile([128, KC, DLOC], F32R, tag="t_wk")
            t_wv = cpool.tile([128, KC, DLOC], F32R, tag="t_wv")
            t_wp = cpool.tile([128, 2, D], F32R, tag="t_wp")
            t_mask = cpool.tile([128, 4, 1024], F32R, tag="t_mask")
            t_bq = cpool.tile([128, 2], F32, tag="t_bq")
            t_bk = cpool.tile([128, 2], F32, tag="t_bk")
            t_ones = cpool.tile([32, 128], F32R, tag="t_ones")
            t_bvp = cpool.tile([32, DLOC], F32R, tag="t_bvp")
            t_qt = cpool.tile([128, 2, T], F32R, tag="t_qt")
            t_kt = cpool.tile([128, 2, T], F32R, tag="t_kt")
            t_v = cpool.tile([128, QC, HLOC, HD + 1], F32R, tag="t_v")
            t_ot = cpool.tile([128, 2, T], F32R, tag="t_ot")
            t_onesm = cpool.tile([128, 4, 64], F32R, tag="t_onesm")
            t_rec4 = cpool.tile([128, 512], F32R, tag="t_rec4")
            t_sums4 = cpool.tile([128, 512], F32, tag="t_sums4")

            nc.sync.dma_start(t_wq[:], wq_r.bitcast(F32R))
            nc.sync.dma_start(t_wk[:], wk_r.bitcast(F32R))
            nc.sync.dma_start(t_wv[:], wv_r.bitcast(F32R))
            nc.sync.dma_start(t_wp[:], wp_r.bitcast(F32R))
            nc.sync.dma_start(t_mask[:], mask[:].bitcast(F32R))
            nc.sync.dma_start(t_bq[:], bq_r)
            nc.sync.dma_start(t_bk[:], bk_r)
            nc.sync.dma_start(t_ones[:], ones[:].bitcast(F32R))
            nc.sync.dma_start(t_bvp[:], bvp[:].bitcast(F32R))
            nc.sync.dma_start(t_v[:], vinit[:].bitcast(F32R))
            nc.sync.dma_start(t_onesm[:], onesm[:].bitcast(F32R))
            nc.sync.dma_start(t_sums4[:], onesr[:])

            # ---- Phase B: projections ----
            for nt in range(NT):
                c0 = 512 * nt
                t_xp = wpool.tile([128, KC, 512], F32R, tag="t_xp")
                nc.sync.dma_start(t_xp[:], xpt_r[:, :, c0 : c0 + 512].bitcast(F32R))
                for t_w, t_b, t_dst in ((t_wq, t_bq, t_qt), (t_wk, t_bk, t_kt)):
                    for m in range(2):
                        p = psB.tile([128, 2, 512], F32, tag="st")
                        for k in range(KC):
                            nc.tensor.matmul(
                                p[:, 0, :],
                                t_w[:, k, 128 * m : 128 * m + 128],
                                t_xp[:, k, :],
                                start=(k == 0),
                                stop=(k == KC - 1),
                            )
                        nc.scalar.activation(
                            t_dst[:, m, c0 : c0 + 512],
                            p[:, 0, :],
                            AF.Identity,
                            bias=t_b[:, m : m + 1],
                        )
                for tc4 in range(4):
                    tch = 4 * nt + tc4
                    p = psB.tile([128, 2, 512], F32, tag="st")
                    for k in range(KC):
                        nc.tensor.matmul(
                            p[:, 0, :DLOC],
                            t_xp[:, k, 128 * tc4 : 128 * tc4 + 128],
                            t_wv[:, k, :],
                            start=(k == 0),
                            stop=False,
                        )
                    nc.tensor.matmul(
                        p[:, 0, :DLOC], t_ones[:], t_bvp[:], start=False, stop=True
                    )
                    nc.vector.tensor_copy(
                        out=t_v[:, tch, :, 0:HD],
                        in_=p[:, 0, :DLOC].rearrange("p (h d) -> p h d", h=HLOC),
                    )

            # ---- Phase C + D: attention (head-pair row-tiled) and output proj ----
            # Per (qt, hf): both heads of a pair do score blocks as
            # row-tiled concurrent K=64 matmuls (tile_position rows 0/64),
            # one exp per k-block covers the pair, emission is software-
            # pipelined (OT of block j lands after ST of block j+1).
            for qt in range(NT):
                q0 = 512 * qt
                nblk = 4 * qt + 4
                for hf in range(2):
                    ot_pair = [
                        psC.tile([65, 512], F32, tag="ot", name=f"ot_{qt}_{hf}_{pp}")
                        for pp in range(2)
                    ]
                    exs = {}

                    def emit_st(j):
                        st = psB.tile([128, 2, 512], F32, tag="st")
                        for pp in range(2):
                            hp = 64 * pp
                            nc.tensor.matmul(
                                st[:, pp, :],
                                t_kt[hp : hp + 64, hf, 128 * j : 128 * j + 128],
                                t_qt[hp : hp + 64, hf, q0 : q0 + 512],
                                start=True,
                                stop=True,
                            )
                        ex = epool.tile([128, 2, 512], F32R, tag="ex")
                        nc.scalar.activation(ex[:], st[:], AF.Exp, scale=float(SCALE))
                        if j >= 4 * qt:
                            m = j - 4 * qt
                            nc.vector.tensor_tensor(
                                ex[:],
                                ex[:],
                                t_mask[:, m, :].rearrange("p (two n) -> p two n", two=2),
                                MUL,
                            )
                        exs[j] = ex

                    def emit_ot(j):
                        ex = exs.pop(j)
                        for pp in range(2):
                            nc.tensor.matmul(
                                ot_pair[pp][:],
                                t_v[:, j, 2 * hf + pp, :],
                                ex[:, pp, :],
                                start=(j == 0),
                                stop=(j == nblk - 1),
                            )

                    emit_st(0)
                    for j in range(1, nblk):
                        emit_st(j)
                        emit_ot(j - 1)
                    emit_ot(nblk - 1)

                    # stage unnormalized OT to SBUF (frees the PSUM slot fast)
                    pair_stages = []
                    for pp in range(2):
                        stage = wpool.tile([65, 512], F32, tag="stg")
                        nc.vector.tensor_copy(out=stage[:], in_=ot_pair[pp][:])
                        pair_stages.append(stage)
                        # softmax denominators (row 64) onto partition 64*pp
                        nc.vector.tensor_copy(
                            out=t_sums4[64 * pp : 64 * pp + 1, :], in_=stage[64:65, :]
                        )
                    # partition-batched reciprocal for the head pair
                    nc.vector.reciprocal(t_rec4[:], t_sums4[:])
                    for pp in range(2):
                        hp = 64 * pp
                        bc = psB.tile([128, 2, 512], F32, tag="st")
                        nc.tensor.matmul(
                            bc[0:64, 0, :],
                            t_onesm[64 * pp : 64 * pp + 32, pp, :],
                            t_rec4[64 * pp : 64 * pp + 32, :],
                            start=True,
                            stop=True,
                        )
                        nc.vector.tensor_tensor(
                            t_ot[hp : hp + 64, hf, q0 : q0 + 512],
                            bc[0:64, 0, :],
                            pair_stages[pp][0:64, :],
                            MUL,
                        )

                # output projection for the q-chunks this qt completed
                for qc in range(4 * qt, 4 * qt + 4):
                    ty = wpool.tile([128, D], F32, tag="ty")
                    for n2 in range(2):
                        py = psB.tile([128, 2, 512], F32, tag="st")
                        for c in range(2):
                            nc.tensor.matmul(
                                py[:, 0, :],
                                t_ot[:, c, 128 * qc : 128 * qc + 128],
                                t_wp[:, c, 512 * n2 : 512 * n2 + 512],
                                start=(c == 0),
                                stop=(c == 1),
                            )
                        nc.vector.tensor_copy(
                            out=ty[:, 512 * n2 : 512 * n2 + 512], in_=py[:, 0, :]
                        )
                    nc.sync.dma_start(y[128 * qc : 128 * qc + 128, :], ty[:])

    nc.compile()
    return nc


def _get_nc():
    if "nc" not in _CACHE:
        _CACHE["nc"] = _build()
    return _CACHE["nc"]


def _make_in_maps(xp, Wq, bq, Wk, bk, Wv, bv, Wp, bp):
    xp = np.asarray(xp, np.float32)
    Wq, Wk, Wv, Wp = (np.asarray(a, np.float32) for a in (Wq, Wk, Wv, Wp))
    bq, bk, bv, bp = (np.asarray(a, np.float32) for a in (bq, bk, bv, bp))

    maskv = np.zeros((128, 4, 1024), np.float32)
    for m in range(4):
        for p in range(128):
            maskv[p, m, 128 * m + p : 512] = 1.0
            maskv[p, m, 512 + 128 * m + p :] = 1.0
    onesv = np.zeros((32, 128), np.float32)
    onesv[0] = 1.0
    onesmv = np.zeros((128, 4, 64), np.float32)
    for pp in range(2):
        onesmv[64 * pp, pp, :] = 1.0
    onesrv = np.ones((128, 512), np.float32)
    vinitv = np.zeros((128, QC, HLOC, HD + 1), np.float32)
    vinitv[:, :, :, HD] = 1.0

    in_maps = []
    for c in range(8):
        b, g = divmod(c, 4)
        s = slice(DLOC * g, DLOC * (g + 1))
        bvpv = np.zeros((32, DLOC), np.float32)
        bvpv[0] = bv[s]
        in_maps.append(
            {
                "xpt": np.ascontiguousarray(xp[b].T),
                "wq": np.ascontiguousarray(Wq[:, s]),
                "wk": np.ascontiguousarray(Wk[:, s]),
                "wv": np.ascontiguousarray(Wv[:, s]),
                "bq": np.ascontiguousarray(bq[s]),
                "bk": np.ascontiguousarray(bk[s]),
                "bvp": bvpv,
                "wp": np.ascontiguousarray(Wp[s, :]),
                "mask": maskv,
                "ones": onesv,
                "onesm": onesmv,
                "vinit": vinitv,
                "onesr": onesrv,
            }
        )

    return in_maps


def _gather(results, bp):
    out = np.zeros((B, T, D), np.float32)
    for c in range(8):
        out[c // 4] += results[c]["y"]
    out += np.asarray(bp, np.float32)[None, None, :]
    return out


def kernel(xp, Wq, bq, Wk, bk, Wv, bv, Wp, bp):
    nc = _get_nc()
    in_maps = _make_in_maps(xp, Wq, bq, Wk, bk, Wv, bv, Wp, bp)
    res = run_bass_kernel_spmd(nc, in_maps, list(range(8)))
    return _gather(res.results, bp)


# revision 13
# speedup vs baseline: 1.1438x; 1.1438x over previous
"""Causal self-attention (B=2, T=2048, D_in=1152, D=1024, H=16) on 8 trn2 cores.

Sharding: 2-way data parallel over batch x 4-way tensor parallel over heads.
Core c handles batch b = c//4 and heads [4g, 4g+4) with g = c%4.

Per-core dataflow (all matmuls in float32r, ~fp32 precision at bf16 speed):
  QT = (Wq_g)^T @ xp[b]^T   -> [256, 2048]   (head dims on partitions)
  KT likewise; V = xp[b] @ Wv_g in natural [T, 256] layout (T on partitions),
  stored with a ones-column per head: Vh~ = [V_h | 1] as [128, 16, 4, 65].
  Scores transposed: ST[k, q] = K Q^T built per 128-row k-block so softmax
  denominators come free: OT~ = Vh~^T @ exp(ST/8) accumulates [65, 512] in
  PSUM where row 64 is the softmax row-sum. exp is unshifted (scores are
  N(0,1) after scaling - safe in fp32). Causal mask = 0/1 multiply after exp
  on diagonal blocks only; fully-masked blocks are skipped.
  Normalize via reciprocal + rank-1 (K=32 zero-padded) broadcast matmul,
  write into OT_all [256, 2048], then Y_partial = OT_all^T @ Wp_g.
Host sums the 4 partial Y per batch and adds bp.
"""

import numpy as np

import concourse.bass as bass
import concourse.mybir as mybir
import concourse.tile as tile
from concourse import bacc
from concourse.bass_utils import run_bass_kernel_spmd

F32 = mybir.dt.float32
F32R = mybir.dt.float32r
AF = mybir.ActivationFunctionType
MUL = mybir.AluOpType.mult

B, T, DIN, D, H = 2, 2048, 1152, 1024, 16
HD = D // H           # 64 head dim
HLOC = 4              # heads per core
DLOC = HLOC * HD      # 256 local model dims
KC = DIN // 128       # 9 contraction chunks for projections
NT = T // 512         # 4 column tiles of 512
QC = T // 128         # 16 row chunks of 128
SCALE = 1.0 / np.sqrt(np.float32(HD))

_CACHE = {}


def _build():
    nc = bacc.Bacc(None)

    xpt = nc.dram_tensor("xpt", [DIN, T], F32, kind="ExternalInput")
    wq = nc.dram_tensor("wq", [DIN, DLOC], F32, kind="ExternalInput")
    wk = nc.dram_tensor("wk", [DIN, DLOC], F32, kind="ExternalInput")
    wv = nc.dram_tensor("wv", [DIN, DLOC], F32, kind="ExternalInput")
    bq = nc.dram_tensor("bq", [DLOC], F32, kind="ExternalInput")
    bk = nc.dram_tensor("bk", [DLOC], F32, kind="ExternalInput")
    bvp = nc.dram_tensor("bvp", [32, DLOC], F32, kind="ExternalInput")
    wp = nc.dram_tensor("wp", [DLOC, D], F32, kind="ExternalInput")
    mask = nc.dram_tensor("mask", [128, 4, 1024], F32, kind="ExternalInput")
    ones = nc.dram_tensor("ones", [32, 128], F32, kind="ExternalInput")
    onesm = nc.dram_tensor("onesm", [128, 4, 64], F32, kind="ExternalInput")
    vinit = nc.dram_tensor("vinit", [128, QC, HLOC, HD + 1], F32, kind="ExternalInput")
    onesr = nc.dram_tensor("onesr", [128, 512], F32, kind="ExternalInput")
    y = nc.dram_tensor("y", [T, D], F32, kind="ExternalOutput")

    xpt_r = xpt.rearrange("(ko p) t -> p ko t", p=128)
    wq_r = wq.rearrange("(ko p) d -> p ko d", p=128)
    wk_r = wk.rearrange("(ko p) d -> p ko d", p=128)
    wv_r = wv.rearrange("(ko p) d -> p ko d", p=128)
    wp_r = wp.rearrange("(c p) n -> p c n", p=128)
    bq_r = bq.rearrange("(m p) -> p m", p=128)
    bk_r = bk.rearrange("(m p) -> p m", p=128)

    with tile.TileContext(nc) as tc:
        with (
            tc.tile_pool(name="const", bufs=1) as cpool,
            tc.tile_pool(name="work", bufs=2) as wpool,
            tc.tile_pool(name="exp", bufs=4) as epool,
            tc.tile_pool(name="psB", bufs=2, space="PSUM") as psB,
            tc.tile_pool(name="psC", bufs=2, space="PSUM") as psC,
            tc.tile_pool(name="psX", bufs=2, space="PSUM") as psX,
            nc.allow_low_precision(reason="float32r matmul pipeline"),
        ):
            t_wq = cpool.tile([128, KC, DLOC], F32R, tag="t_wq")
            t_wk = cpool.tile([128, KC, DLOC], F32R, tag="t_wk")
            t_wv = cpool.tile([128, KC, DLOC], F32R, tag="t_wv")
            t_wp = cpool.tile([128, 2, D], F32R, tag="t_wp")
            t_mask = cpool.tile([128, 4, 1024], F32R, tag="t_mask")
            t_bq = cpool.tile([128, 2], F32, tag="t_bq")
            t_bk = cpool.tile([128, 2], F32, tag="t_bk")
            t_ones = cpool.tile([32, 128], F32R, tag="t_ones")
            t_bvp = cpool.tile([32, DLOC], F32R, tag="t_bvp")
            t_qt = cpool.tile([128, 2, T], F32R, tag="t_qt")
            t_kt = cpool.tile([128, 2, T], F32R, tag="t_kt")
            t_v = cpool.tile([128, QC, HLOC, HD + 1], F32R, tag="t_v")
            t_ot = cpool.tile([128, 2, T], F32R, tag="t_ot")
            t_onesm = cpool.tile([128, 4, 64], F32R, tag="t_onesm")
            t_rec4 = cpool.tile([128, 512], F32R, tag="t_rec4")
            t_sums4 = cpool.tile([128, 512], F32, tag="t_sums4")

            nc.sync.dma_start(t_wq[:], wq_r.bitcast(F32R))
            nc.sync.dma_start(t_wk[:], wk_r.bitcast(F32R))
            nc.sync.dma_start(t_wv[:], wv_r.bitcast(F32R))
            nc.sync.dma_start(t_wp[:], wp_r.bitcast(F32R))
            nc.sync.dma_start(t_mask[:], mask[:].bitcast(F32R))
            nc.sync.dma_start(t_bq[:], bq_r)
            nc.sync.dma_start(t_bk[:], bk_r)
            nc.sync.dma_start(t_ones[:], ones[:].bitcast(F32R))
            nc.sync.dma_start(t_bvp[:], bvp[:].bitcast(F32R))
            nc.sync.dma_start(t_v[:], vinit[:].bitcast(F32R))
            nc.sync.dma_start(t_onesm[:], onesm[:].bitcast(F32R))
            nc.sync.dma_start(t_sums4[:], onesr[:])

            # ---- Phase B: projections ----
            for nt in range(NT):
                c0 = 512 * nt
                t_xp = wpool.tile([128, KC, 512], F32R, tag="t_xp")
                nc.sync.dma_start(t_xp[:], xpt_r[:, :, c0 : c0 + 512].bitcast(F32R))
                for t_w, t_b, t_dst in ((t_wq, t_bq, t_qt), (t_wk, t_bk, t_kt)):
                    for m in range(2):
                        p = psB.tile([128, 2, 512], F32, tag="st")
                        for k in range(KC):
                            nc.tensor.matmul(
                                p[:, 0, :],
                                t_w[:, k, 128 * m : 128 * m + 128],
                                t_xp[:, k, :],
                                start=(k == 0),
                                stop=(k == KC - 1),
                            )
                        nc.scalar.activation(
                            t_dst[:, m, c0 : c0 + 512],
                            p[:, 0, :],
                            AF.Identity,
                            bias=t_b[:, m : m + 1],
                        )
                for tc4 in range(4):
                    tch = 4 * nt + tc4
                    p = psB.tile([128, 2, 512], F32, tag="st")
                    for k in range(KC):
                        nc.tensor.matmul(
                            p[:, 0, :DLOC],
                            t_xp[:, k, 128 * tc4 : 128 * tc4 + 128],
                            t_wv[:, k, :],
                            start=(k == 0),
                            stop=False,
                        )
                    nc.tensor.matmul(
                        p[:, 0, :DLOC], t_ones[:], t_bvp[:], start=False, stop=True
                    )
                    nc.vector.tensor_copy(
                        out=t_v[:, tch, :, 0:HD],
                        in_=p[:, 0, :DLOC].rearrange("p (h d) -> p h d", h=HLOC),
                    )

            # ---- Phase C + D: attention (head-pair row-tiled) and output proj ----
            # Per (qt, hf): both heads of a pair do score blocks as
            # row-tiled concurrent K=64 matmuls (tile_position rows 0/64),
            # one exp per k-block covers the pair, emission is software-
            # pipelined (OT of block j lands after ST of block j+1).
            for qt in range(NT):
                q0 = 512 * qt
                nblk = 4 * qt + 4
                for hf in range(2):
                    ot_pair = [
                        psC.tile([65, 512], F32, tag="ot", name=f"ot_{qt}_{hf}_{pp}")
                        for pp in range(2)
                    ]
                    exs = {}

                    def emit_st(j):
                        st = psB.tile([128, 2, 512], F32, tag="st")
                        for pp in range(2):
                            hp = 64 * pp
                            nc.tensor.matmul(
                                st[:, pp, :],
                                t_kt[hp : hp + 64, hf, 128 * j : 128 * j + 128],
                                t_qt[hp : hp + 64, hf, q0 : q0 + 512],
                                start=True,
                                stop=True,
                            )
                        ex = epool.tile([128, 2, 512], F32R, tag="ex")
                        nc.scalar.activation(ex[:], st[:], AF.Exp, scale=float(SCALE))
                        if j >= 4 * qt:
                            m = j - 4 * qt
                            nc.vector.tensor_tensor(
                                ex[:],
                                ex[:],
                                t_mask[:, m, :].rearrange("p (two n) -> p two n", two=2),
                                MUL,
                            )
                        exs[j] = ex

                    def emit_ot(j):
                        ex = exs.pop(j)
                        for pp in range(2):
                            nc.tensor.matmul(
                                ot_pair[pp][:],
                                t_v[:, j, 2 * hf + pp, :],
                                ex[:, pp, :],
                                start=(j == 0),
                                stop=(j == nblk - 1),
                            )

                    emit_st(0)
                    for j in range(1, nblk):
                        emit_st(j)
                        emit_ot(j - 1)
                    emit_ot(nblk - 1)

                    # stage unnormalized OT to SBUF (frees the PSUM slot fast)
                    pair_stages = []
                    for pp in range(2):
                        stage = wpool.tile([65, 512], F32, tag="stg")
                        nc.vector.tensor_copy(out=stage[:], in_=ot_pair[pp][:])
                        pair_stages.append(stage)
                        # softmax denominators (row 64) onto partition 64*pp
                        nc.vector.tensor_copy(
                            out=t_sums4[64 * pp : 64 * pp + 1, :], in_=stage[64:65, :]
                        )
                    # partition-batched reciprocal for the head pair
                    nc.vector.reciprocal(t_rec4[:], t_sums4[:])
                    for pp in range(2):
                        hp = 64 * pp
                        bc = psX.tile([128, 512], F32, tag="aux")
                        nc.tensor.matmul(
                            bc[0:64, :],
                            t_onesm[64 * pp : 64 * pp + 32, pp, :],
                            t_rec4[64 * pp : 64 * pp + 32, :],
                            start=True,
                            stop=True,
                        )
                        nc.vector.tensor_tensor(
                            t_ot[hp : hp + 64, hf, q0 : q0 + 512],
                            bc[0:64, :],
                            pair_stages[pp][0:64, :],
                            MUL,
                        )

                # output projection for the q-chunks this qt completed
                for qc in range(4 * qt, 4 * qt + 4):
                    ty = wpool.tile([128, D], F32, tag="ty")
                    for n2 in range(2):
                        py = psX.tile([128, 512], F32, tag="aux")
                        for c in range(2):
                            nc.tensor.matmul(
                                py[:],
                                t_ot[:, c, 128 * qc : 128 * qc + 128],
                                t_wp[:, c, 512 * n2 : 512 * n2 + 512],
                                start=(c == 0),
                                stop=(c == 1),
                            )
                        nc.vector.tensor_copy(
                            out=ty[:, 512 * n2 : 512 * n2 + 512], in_=py[:]
                        )
                    nc.sync.dma_start(y[128 * qc : 128 * qc + 128, :], ty[:])

    nc.compile()
    return nc


def _get_nc():
    if "nc" not in _CACHE:
        _CACHE["nc"] = _build()
    return _CACHE["nc"]


def _make_in_maps(xp, Wq, bq, Wk, bk, Wv, bv, Wp, bp):
    xp = np.asarray(xp, np.float32)
    Wq, Wk, Wv, Wp = (np.asarray(a, np.float32) for a in (Wq, Wk, Wv, Wp))
    bq, bk, bv, bp = (np.asarray(a, np.float32) for a in (bq, bk, bv, bp))

    maskv = np.zeros((128, 4, 1024), np.float32)
    for m in range(4):
        for p in range(128):
            maskv[p, m, 128 * m + p : 512] = 1.0
            maskv[p, m, 512 + 128 * m + p :] = 1.0
    onesv = np.zeros((32, 128), np.float32)
    onesv[0] = 1.0
    onesmv = np.zeros((128, 4, 64), np.float32)
    for pp in range(2):
        onesmv[64 * pp, pp, :] = 1.0
    onesrv = np.ones((128, 512), np.float32)
    vinitv = np.zeros((128, QC, HLOC, HD + 1), np.float32)
    vinitv[:, :, :, HD] = 1.0

    in_maps = []
    for c in range(8):
        b, g = divmod(c, 4)
        s = slice(DLOC * g, DLOC * (g + 1))
        bvpv = np.zeros((32, DLOC), np.float32)
        bvpv[0] = bv[s]
        in_maps.append(
            {
                "xpt": np.ascontiguousarray(xp[b].T),
                "wq": np.ascontiguousarray(Wq[:, s]),
                "wk": np.ascontiguousarray(Wk[:, s]),
                "wv": np.ascontiguousarray(Wv[:, s]),
                "bq": np.ascontiguousarray(bq[s]),
                "bk": np.ascontiguousarray(bk[s]),
                "bvp": bvpv,
                "wp": np.ascontiguousarray(Wp[s, :]),
                "mask": maskv,
                "ones": onesv,
                "onesm": onesmv,
                "vinit": vinitv,
                "onesr": onesrv,
            }
        )

    return in_maps


def _gather(results, bp):
    out = np.zeros((B, T, D), np.float32)
    for c in range(8):
        out[c // 4] += results[c]["y"]
    out += np.asarray(bp, np.float32)[None, None, :]
    return out


def kernel(xp, Wq, bq, Wk, bk, Wv, bv, Wp, bp):
    nc = _get_nc()
    in_maps = _make_in_maps(xp, Wq, bq, Wk, bk, Wv, bv, Wp, bp)
    res = run_bass_kernel_spmd(nc, in_maps, list(range(8)))
    return _gather(res.results, bp)


# revision 14
# speedup vs baseline: 1.1682x; 1.0213x over previous
"""Causal self-attention (B=2, T=2048, D_in=1152, D=1024, H=16) on 8 trn2 cores.

Sharding: 2-way data parallel over batch x 4-way tensor parallel over heads.
Core c handles batch b = c//4 and heads [4g, 4g+4) with g = c%4.

Per-core dataflow (all matmuls in float32r, ~fp32 precision at bf16 speed):
  QT = (Wq_g)^T @ xp[b]^T   -> [256, 2048]   (head dims on partitions)
  KT likewise; V = xp[b] @ Wv_g in natural [T, 256] layout (T on partitions),
  stored with a ones-column per head: Vh~ = [V_h | 1] as [128, 16, 4, 65].
  Scores transposed: ST[k, q] = K Q^T built per 128-row k-block so softmax
  denominators come free: OT~ = Vh~^T @ exp(ST/8) accumulates [65, 512] in
  PSUM where row 64 is the softmax row-sum. exp is unshifted (scores are
  N(0,1) after scaling - safe in fp32). Causal mask = 0/1 multiply after exp
  on diagonal blocks only; fully-masked blocks are skipped.
  Normalize via reciprocal + rank-1 (K=32 zero-padded) broadcast matmul,
  write into OT_all [256, 2048], then Y_partial = OT_all^T @ Wp_g.
Host sums the 4 partial Y per batch and adds bp.
"""

import numpy as np

import concourse.bass as bass
import concourse.mybir as mybir
import concourse.tile as tile
from concourse import bacc
from concourse.bass_utils import run_bass_kernel_spmd

F32 = mybir.dt.float32
F32R = mybir.dt.float32r
AF = mybir.ActivationFunctionType
MUL = mybir.AluOpType.mult

B, T, DIN, D, H = 2, 2048, 1152, 1024, 16
HD = D // H           # 64 head dim
HLOC = 4              # heads per core
DLOC = HLOC * HD      # 256 local model dims
KC = DIN // 128       # 9 contraction chunks for projections
NT = T // 512         # 4 column tiles of 512
QC = T // 128         # 16 row chunks of 128
SCALE = 1.0 / np.sqrt(np.float32(HD))

_CACHE = {}


def _build():
    nc = bacc.Bacc(None)

    xpt = nc.dram_tensor("xpt", [DIN, T], F32, kind="ExternalInput")
    wq = nc.dram_tensor("wq", [DIN, DLOC], F32, kind="ExternalInput")
    wk = nc.dram_tensor("wk", [DIN, DLOC], F32, kind="ExternalInput")
    wv = nc.dram_tensor("wv", [DIN, DLOC], F32, kind="ExternalInput")
    bq = nc.dram_tensor("bq", [DLOC], F32, kind="ExternalInput")
    bk = nc.dram_tensor("bk", [DLOC], F32, kind="ExternalInput")
    bvp = nc.dram_tensor("bvp", [32, DLOC], F32, kind="ExternalInput")
    wp = nc.dram_tensor("wp", [DLOC, D], F32, kind="ExternalInput")
    mask = nc.dram_tensor("mask", [128, 4, 1024], F32, kind="ExternalInput")
    ones = nc.dram_tensor("ones", [32, 128], F32, kind="ExternalInput")
    onesm = nc.dram_tensor("onesm", [128, 4, 64], F32, kind="ExternalInput")
    vinit = nc.dram_tensor("vinit", [128, QC, HLOC, HD + 1], F32, kind="ExternalInput")
    onesr = nc.dram_tensor("onesr", [128, 512], F32, kind="ExternalInput")
    y = nc.dram_tensor("y", [T, D], F32, kind="ExternalOutput")

    xpt_r = xpt.rearrange("(ko p) t -> p ko t", p=128)
    wq_r = wq.rearrange("(ko p) d -> p ko d", p=128)
    wk_r = wk.rearrange("(ko p) d -> p ko d", p=128)
    wv_r = wv.rearrange("(ko p) d -> p ko d", p=128)
    wp_r = wp.rearrange("(c p) n -> p c n", p=128)
    bq_r = bq.rearrange("(m p) -> p m", p=128)
    bk_r = bk.rearrange("(m p) -> p m", p=128)

    with tile.TileContext(nc) as tc:
        with (
            tc.tile_pool(name="const", bufs=1) as cpool,
            tc.tile_pool(name="work", bufs=2) as wpool,
            tc.tile_pool(name="exp", bufs=4) as epool,
            tc.tile_pool(name="psB", bufs=2, space="PSUM") as psB,
            tc.tile_pool(name="psC", bufs=2, space="PSUM") as psC,
            tc.tile_pool(name="psX", bufs=2, space="PSUM") as psX,
            nc.allow_low_precision(reason="float32r matmul pipeline"),
        ):
            t_wq = cpool.tile([128, KC, DLOC], F32R, tag="t_wq")
            t_wk = cpool.tile([128, KC, DLOC], F32R, tag="t_wk")
            t_wv = cpool.tile([128, KC, DLOC], F32R, tag="t_wv")
            t_wp = cpool.tile([128, 2, D], F32R, tag="t_wp")
            t_mask = cpool.tile([128, 4, 1024], F32R, tag="t_mask")
            t_bq = cpool.tile([128, 2], F32, tag="t_bq")
            t_bk = cpool.tile([128, 2], F32, tag="t_bk")
            t_ones = cpool.tile([32, 128], F32R, tag="t_ones")
            t_bvp = cpool.tile([32, DLOC], F32R, tag="t_bvp")
            t_qt = cpool.tile([128, 2, T], F32R, tag="t_qt")
            t_kt = cpool.tile([128, 2, T], F32R, tag="t_kt")
            t_v = cpool.tile([128, QC, HLOC, HD + 1], F32R, tag="t_v")
            t_ot = cpool.tile([128, 2, T], F32R, tag="t_ot")
            t_onesm = cpool.tile([128, 4, 64], F32R, tag="t_onesm")
            t_rec4 = cpool.tile([128, 512], F32R, tag="t_rec4")
            t_sums4 = cpool.tile([128, 512], F32, tag="t_sums4")

            nc.sync.dma_start(t_wq[:], wq_r.bitcast(F32R))
            nc.sync.dma_start(t_wk[:], wk_r.bitcast(F32R))
            nc.sync.dma_start(t_wv[:], wv_r.bitcast(F32R))
            nc.sync.dma_start(t_bq[:], bq_r)
            nc.sync.dma_start(t_bk[:], bk_r)
            nc.sync.dma_start(t_ones[:], ones[:].bitcast(F32R))
            nc.sync.dma_start(t_bvp[:], bvp[:].bitcast(F32R))
            nc.sync.dma_start(t_v[:], vinit[:].bitcast(F32R))
            nc.sync.dma_start(t_mask[:], mask[:].bitcast(F32R))
            nc.sync.dma_start(t_wp[:], wp_r.bitcast(F32R))
            nc.sync.dma_start(t_onesm[:], onesm[:].bitcast(F32R))
            nc.sync.dma_start(t_sums4[:], onesr[:])

            def proj(nt):
                c0 = 512 * nt
                t_xp = wpool.tile([128, KC, 512], F32R, tag="t_xp")
                nc.sync.dma_start(t_xp[:], xpt_r[:, :, c0 : c0 + 512].bitcast(F32R))
                for t_w, t_b, t_dst in ((t_wq, t_bq, t_qt), (t_wk, t_bk, t_kt)):
                    for m in range(2):
                        p = psB.tile([128, 2, 512], F32, tag="st")
                        for k in range(KC):
                            nc.tensor.matmul(
                                p[:, 0, :],
                                t_w[:, k, 128 * m : 128 * m + 128],
                                t_xp[:, k, :],
                                start=(k == 0),
                                stop=(k == KC - 1),
                            )
                        nc.scalar.activation(
                            t_dst[:, m, c0 : c0 + 512],
                            p[:, 0, :],
                            AF.Identity,
                            bias=t_b[:, m : m + 1],
                        )
                for tc4 in range(4):
                    tch = 4 * nt + tc4
                    p = psB.tile([128, 2, 512], F32, tag="st")
                    for k in range(KC):
                        nc.tensor.matmul(
                            p[:, 0, :DLOC],
                            t_xp[:, k, 128 * tc4 : 128 * tc4 + 128],
                            t_wv[:, k, :],
                            start=(k == 0),
                            stop=False,
                        )
                    nc.tensor.matmul(
                        p[:, 0, :DLOC], t_ones[:], t_bvp[:], start=False, stop=True
                    )
                    nc.vector.tensor_copy(
                        out=t_v[:, tch, :, 0:HD],
                        in_=p[:, 0, :DLOC].rearrange("p (h d) -> p h d", h=HLOC),
                    )

            # ---- Interleaved: proj(nt) then attention+output-proj for qt=nt.
            # attention(qt) only reads projection columns <= 512*(qt+1), all
            # produced by proj(0..qt), so the engines pipeline across phases.
            proj(0)
            for qt in range(NT):
                if qt + 1 < NT:
                    proj(qt + 1)
                q0 = 512 * qt
                nblk = 4 * qt + 4
                for hf in range(2):
                    ot_pair = [
                        psC.tile([65, 512], F32, tag="ot", name=f"ot_{qt}_{hf}_{pp}")
                        for pp in range(2)
                    ]
                    exs = {}

                    def emit_st(j):
                        st = psB.tile([128, 2, 512], F32, tag="st")
                        for pp in range(2):
                            hp = 64 * pp
                            nc.tensor.matmul(
                                st[:, pp, :],
                                t_kt[hp : hp + 64, hf, 128 * j : 128 * j + 128],
                                t_qt[hp : hp + 64, hf, q0 : q0 + 512],
                                start=True,
                                stop=True,
                            )
                        ex = epool.tile([128, 2, 512], F32R, tag="ex")
                        nc.scalar.activation(ex[:], st[:], AF.Exp, scale=float(SCALE))
                        if j >= 4 * qt:
                            m = j - 4 * qt
                            nc.vector.tensor_tensor(
                                ex[:],
                                ex[:],
                                t_mask[:, m, :].rearrange("p (two n) -> p two n", two=2),
                                MUL,
                            )
                        exs[j] = ex

                    def emit_ot(j):
                        ex = exs.pop(j)
                        for pp in range(2):
                            nc.tensor.matmul(
                                ot_pair[pp][:],
                                t_v[:, j, 2 * hf + pp, :],
                                ex[:, pp, :],
                                start=(j == 0),
                                stop=(j == nblk - 1),
                            )

                    emit_st(0)
                    for j in range(1, nblk):
                        emit_st(j)
                        emit_ot(j - 1)
                    emit_ot(nblk - 1)

                    # stage unnormalized OT to SBUF (frees the PSUM slot fast)
                    pair_stages = []
                    for pp in range(2):
                        stage = wpool.tile([65, 512], F32, tag="stg")
                        nc.vector.tensor_copy(out=stage[:], in_=ot_pair[pp][:])
                        pair_stages.append(stage)
                        # softmax denominators (row 64) onto partition 64*pp
                        nc.vector.tensor_copy(
                            out=t_sums4[64 * pp : 64 * pp + 1, :], in_=stage[64:65, :]
                        )
                    # partition-batched reciprocal for the head pair
                    nc.vector.reciprocal(t_rec4[:], t_sums4[:])
                    for pp in range(2):
                        hp = 64 * pp
                        bc = psX.tile([128, 512], F32, tag="aux")
                        nc.tensor.matmul(
                            bc[0:64, :],
                            t_onesm[64 * pp : 64 * pp + 32, pp, :],
                            t_rec4[64 * pp : 64 * pp + 32, :],
                            start=True,
                            stop=True,
                        )
                        nc.vector.tensor_tensor(
                            t_ot[hp : hp + 64, hf, q0 : q0 + 512],
                            bc[0:64, :],
                            pair_stages[pp][0:64, :],
                            MUL,
                        )

                # output projection for the q-chunks this qt completed
                for qc in range(4 * qt, 4 * qt + 4):
                    ty = wpool.tile([128, D], F32, tag="ty")
                    for n2 in range(2):
                        py = psX.tile([128, 512], F32, tag="aux")
                        for c in range(2):
                            nc.tensor.matmul(
                                py[:],
                                t_ot[:, c, 128 * qc : 128 * qc + 128],
                                t_wp[:, c, 512 * n2 : 512 * n2 + 512],
                                start=(c == 0),
                                stop=(c == 1),
                            )
                        nc.vector.tensor_copy(
                            out=ty[:, 512 * n2 : 512 * n2 + 512], in_=py[:]
                        )
                    nc.sync.dma_start(y[128 * qc : 128 * qc + 128, :], ty[:])

    nc.compile()
    return nc


def _get_nc():
    if "nc" not in _CACHE:
        _CACHE["nc"] = _build()
    return _CACHE["nc"]


def _make_in_maps(xp, Wq, bq, Wk, bk, Wv, bv, Wp, bp):
    xp = np.asarray(xp, np.float32)
    Wq, Wk, Wv, Wp = (np.asarray(a, np.float32) for a in (Wq, Wk, Wv, Wp))
    bq, bk, bv, bp = (np.asarray(a, np.float32) for a in (bq, bk, bv, bp))

    maskv = np.zeros((128, 4, 1024), np.float32)
    for m in range(4):
        for p in range(128):
            maskv[p, m, 128 * m + p : 512] = 1.0
            maskv[p, m, 512 + 128 * m + p :] = 1.0
    onesv = np.zeros((32, 128), np.float32)
    onesv[0] = 1.0
    onesmv = np.zeros((128, 4, 64), np.float32)
    for pp in range(2):
        onesmv[64 * pp, pp, :] = 1.0
    onesrv = np.ones((128, 512), np.float32)
    vinitv = np.zeros((128, QC, HLOC, HD + 1), np.float32)
    vinitv[:, :, :, HD] = 1.0

    in_maps = []
    for c in range(8):
        b, g = divmod(c, 4)
        s = slice(DLOC * g, DLOC * (g + 1))
        bvpv = np.zeros((32, DLOC), np.float32)
        bvpv[0] = bv[s]
        in_maps.append(
            {
                "xpt": np.ascontiguousarray(xp[b].T),
                "wq": np.ascontiguousarray(Wq[:, s]),
                "wk": np.ascontiguousarray(Wk[:, s]),
                "wv": np.ascontiguousarray(Wv[:, s]),
                "bq": np.ascontiguousarray(bq[s]),
                "bk": np.ascontiguousarray(bk[s]),
                "bvp": bvpv,
                "wp": np.ascontiguousarray(Wp[s, :]),
                "mask": maskv,
                "ones": onesv,
                "onesm": onesmv,
                "vinit": vinitv,
                "onesr": onesrv,
            }
        )

    return in_maps


def _gather(results, bp):
    out = np.zeros((B, T, D), np.float32)
    for c in range(8):
        out[c // 4] += results[c]["y"]
    out += np.asarray(bp, np.float32)[None, None, :]
    return out


def kernel(xp, Wq, bq, Wk, bk, Wv, bv, Wp, bp):
    nc = _get_nc()
    in_maps = _make_in_maps(xp, Wq, bq, Wk, bk, Wv, bv, Wp, bp)
    res = run_bass_kernel_spmd(nc, in_maps, list(range(8)))
    return _gather(res.results, bp)
